# revision 13
# baseline (speedup 1.0000x reference)
"""CpxRBM translation-invariant log-psi kernel for 8 Trainium2 NeuronCores.

Computes sum(log(cosh(sym @ W.T))) where sym is the (4095, 4096) matrix of
circular shifts of v = 2*vis_states - 1 and W is (1024, 4096) complex64.

v4 strategy (shift-sharded, 512 shifts/core, fp8 DoubleRow matmuls):
  - Host sends the per-core 4608-window of doubled v as fp8 {-1,+1} (exact)
    and the weights as fp8e4 scaled by 64 (power of two; undone for free via
    ACT affine scales).  Core 7's phantom 4096th shift (the wrap-around
    shift 4095) is subtracted EXACTLY on the host using the same
    fp8-dequantized weights.
  - Matmul: perf_mode=DoubleRow packs 2 fp8 weights/PE cell: each matmul
    contracts K=256 (two 128-row chunks) with a 128x1024-fp8 moving operand
    -> 256 matmuls x 512 out cols (216ns each, ~2x over bf16).  sym slices
    stationary, weights moving, fp32 PSUM accumulation.
  - OUTPUT-QUARTER-outer loop: each ~13.8us matmul phase consumes 2.1MB of
    weights (~150GB/s, sustainable).  Weights arrive as 4x524KB DMAs per
    quarter with 4KB-contiguous partition lines (full HBM rate); the host
    pre-swizzles the layout.  The last quarter runs as two half-phases so
    the final elementwise tail is halved.
  - log(cosh(x+iy)) elementwise (x,y read straight from PSUM by ACT):
      sy = sin(y), cy = sin(pi/2 - |y|) = cos(y)   (direct: |y| <= ~3.6 and
        the HW sin table is accurate there - verified by probe; no range
        reduction at all)
      q = (e^x+e^-x)^2 - 4 sy^2 = |2cosh z|^2
      Re = 0.5*ln(q + 1e-6) - ln2        (ACT Ln accumulates row sums)
      Im = atan2(b, a) = atan(b/a) + pi*[a<0]*sign(b),
        a = t1*cy, b = t2*sy; 1/|a| via Exp/Ln (same table set), the
        quadrant counts accumulate via DVE is_lt/is_ge accum_out columns.
        (HW arctan verified accurate over the full input range.)
    Two ACT table sets only (trig_and_small / natural_log_exp_and_others);
    block i's Arctan rides in block i+1's trig residency.
  - Per-core output: (128, 20) fp32 accumulator columns; host reduces.
"""
import math
import numpy as np
import ml_dtypes
from contextlib import ExitStack

import concourse.bass as bass
import concourse.mybir as mybir
import concourse.tile as tile
from concourse import bacc
from concourse.bass_utils import run_bass_kernel_spmd
from concourse.hw_specs import get_activation_tables
import bass_rust as _bass_rust

F32 = mybir.dt.float32
BF16 = mybir.dt.bfloat16
FP8 = mybir.dt.float8e4
AF = mybir.ActivationFunctionType
ALU = mybir.AluOpType
DR = mybir.MatmulPerfMode.DoubleRow

PI = float(np.pi)
VIS_N = 4096
INP_N = 4096
OUP_N = 1024
N_CORES = 8
S_PER_CORE = 512
N_K2 = 16                  # 256-row contraction chunks
N_ST = 4                   # shift tiles of 128 per core
N_Q = 4                    # output quarters
OQ = OUP_N // N_Q          # 256 output cols per quarter
WIN = S_PER_CORE + INP_N   # 4608
SCALE = 64.0               # fp8 weight scale (power of 2)
EPS_Q = 1e-6               # Ln bias: absorbs fp32 rounding of q ~ 0
EPS_A = 1e-4               # Ln bias: absorbs bf16 rounding of |a| ~ 0
# blocks: (quarter, st_lo, st_hi, acc_col_base)
BLOCKS = [(0, 0, 4, 0), (1, 0, 4, 4), (2, 0, 4, 8),
          (3, 0, 2, 12), (3, 2, 4, 16)]
N_COLS = 20

_ALLOWED_SETS = {"natural_log_exp_and_others", "trig_and_small"}


class _Bacc(bacc.Bacc):
    def insert_act_table_loads(self):
        has_activation = any(
            isinstance(i, mybir.InstActivation)
            for b in self.main_func.blocks
            for i in b.instructions
        )
        if not has_activation:
            return
        tables = [
            (name, funcs if name in _ALLOWED_SETS else set())
            for name, funcs in get_activation_tables(self.m.arch).items()
        ]
        _bass_rust.insert_act_table_loads(self, tables)


_nc_cache = None
last_results = None


def _build_nc():
    nc = _Bacc("TRN2", target_bir_lowering=False, debug=False)

    vwin = nc.dram_tensor("vwin", [WIN], FP8, kind="ExternalInput")
    # weights, swizzled: row (q*4 + k2g)*128 + p holds the 4KB line
    # [k2in(4), i(2), j(2), o(256)] for partition p of DMA chunk (q, k2g).
    wq = nc.dram_tensor("wq", [2048, 4096], FP8, kind="ExternalInput")
    acc = nc.dram_tensor("acc", [128, N_COLS], F32, kind="ExternalOutput")

    with tile.TileContext(nc) as tc, ExitStack() as ctx:
        singles = ctx.enter_context(tc.tile_pool(name="singles", bufs=1))
        sympool = ctx.enter_context(tc.tile_pool(name="sympool", bufs=1))
        wpool = ctx.enter_context(tc.tile_pool(name="wpool", bufs=1))
        ppool = ctx.enter_context(tc.tile_pool(name="ppool", bufs=2, space="PSUM"))
        stage = ctx.enter_context(tc.tile_pool(name="stage", bufs=2))

        acc_sb = singles.tile([128, N_COLS], F32)
        half_pi = singles.tile([128, 1], F32)
        nc.vector.memset(half_pi, PI / 2.0)
        eps_q = singles.tile([128, 1], F32)
        nc.vector.memset(eps_q, EPS_Q)
        eps_a = singles.tile([128, 1], F32)
        nc.vector.memset(eps_a, EPS_A)

        # sym windows: symt[k2][p, i, s] = vwin[256*k2 + 128*i + p + s]
        symt = []
        for k2 in range(N_K2):
            st_t = sympool.tile([128, 2, S_PER_CORE], FP8, tag=f"sym{k2}",
                                name=f"sym{k2}")
            nc.scalar.dma_start(
                out=st_t,
                in_=bass.AP(vwin, 256 * k2, [[1, 128], [128, 2], [1, S_PER_CORE]]),
            )
            symt.append(st_t)
        # weights: wt[q][k2g][p, k2in, i, jo]; 524KB per DMA, 4KB/partition
        # contiguous, emitted in consumption order on the sync HWDGE ring.
        wt = [[None] * 4 for _ in range(N_Q)]
        for q in range(N_Q):
            for g in range(4):
                w_t = wpool.tile([128, 4, 2, 2 * OQ], FP8, tag=f"w{q}_{g}",
                                 name=f"w{q}_{g}")
                nc.sync.dma_start(
                    out=w_t,
                    in_=bass.AP(wq, (q * 4 + g) * 128 * 4096,
                                [[4096, 128], [1, 4096]]),
                )
                wt[q][g] = w_t

        state = {"prev_t": None, "prev_col": None, "prev_ops": []}

        def emit_block(ps, st_lo, st_hi, col, last=False):
            """log(cosh) on ps[:, st_lo:st_hi, :] ([128, nst, 512] fp32 psum,
            free cols = [re | im] halves of 256); accumulates into acc_sb
            cols col+0: sum ln q, col+1: sum atan, col+2: sum g, col+3: g&h."""
            nst = st_hi - st_lo
            xr = ps[:, st_lo:st_hi, 0:OQ]
            xi = ps[:, st_lo:st_hi, OQ:2 * OQ]
            shp = [128, nst, OQ]
            tg = f"_{nst}"

            A = stage.tile(shp, F32, tag="A" + tg)    # sy
            B = stage.tile(shp, F32, tag="B" + tg)    # |y| -> sy^2 -> q
            Bc = stage.tile(shp, F32, tag="Bc" + tg)  # cy
            C = stage.tile(shp, F32, tag="C" + tg)    # e^x -> t1^2
            D = stage.tile(shp, F32, tag="D" + tg)    # e^-x -> t2
            Ft = stage.tile(shp, F32, tag="F" + tg)   # t1
            E = stage.tile(shp, F32, tag="E" + tg)    # ln q scratch
            R1 = stage.tile(shp, BF16, tag="R1" + tg)  # 1/|a|
            R2 = stage.tile(shp, BF16, tag="R2" + tg)  # a -> |a|
            R3 = stage.tile(shp, BF16, tag="R3" + tg)  # b -> t0
            R5 = stage.tile(shp, BF16, tag="R5" + tg)  # t''
            G = stage.tile(shp, BF16, tag="G" + tg)    # [a<0]
            H = stage.tile(shp, BF16, tag="H" + tg)    # [b>=0] -> g*h
            # --- ACT trig phase (prev block's Arctan rides along) ---
            trig_ops = []
            if state["prev_t"] is not None:
                pshp = state["prev_t"].shape
                R4 = stage.tile(list(pshp), BF16, tag=f"R4_{pshp[1]}")
                i_at = nc.scalar.activation(
                    R4, state["prev_t"], AF.Arctan,
                    accum_out=acc_sb[:, state["prev_col"]:state["prev_col"] + 1],
                )
                trig_ops.append(i_at)
            i_sy = nc.scalar.activation(A, xi, AF.Sin, scale=1.0 / SCALE)
            i_ay = nc.scalar.activation(B, xi, AF.Abs, scale=1.0 / SCALE)
            i_cy = nc.scalar.activation(Bc, B, AF.Sin, bias=half_pi, scale=-1.0)
            trig_ops += [i_sy, i_ay, i_cy]
            # --- ACT exp phase ---
            i_ep = nc.scalar.activation(C, xr, AF.Exp, scale=1.0 / SCALE)
            i_em = nc.scalar.activation(D, xr, AF.Exp, scale=-1.0 / SCALE)

            # DVE: q chain (fp32: q suffers catastrophic cancellation)
            nc.vector.tensor_tensor(Ft, C, D, ALU.add)           # t1
            nc.vector.tensor_tensor(D, C, D, ALU.subtract)       # t2
            nc.vector.tensor_tensor(C, Ft, Ft, ALU.mult)         # t1^2
            nc.vector.tensor_tensor(B, A, A, ALU.mult)           # sy^2
            nc.vector.scalar_tensor_tensor(B, B, -4.0, C, ALU.mult, ALU.add)
            # DVE: Im chain (bf16)
            nc.vector.tensor_tensor(R2, Ft, Bc, ALU.mult)        # a
            nc.vector.tensor_tensor(R3, D, A, ALU.mult)          # b
            nc.vector.tensor_scalar(G, R2, 0.0, None, ALU.is_lt, ALU.add,
                                    accum_out=acc_sb[:, col + 2:col + 3])
            nc.vector.scalar_tensor_tensor(
                H, R3, 0.0, G, ALU.is_ge, ALU.mult)              # g*[b>=0]
            nc.vector.tensor_scalar(H, H, 1.0, None, ALU.mult, ALU.add,
                                    accum_out=acc_sb[:, col + 3:col + 4])
            nc.vector.scalar_tensor_tensor(R2, R2, -1.0, R2, ALU.mult, ALU.max)

            i_lnq = nc.scalar.activation(
                E, B, AF.Ln, bias=eps_q,
                accum_out=acc_sb[:, col:col + 1])
            i_lna = nc.scalar.activation(E, R2, AF.Ln, bias=eps_a)
            i_inv = nc.scalar.activation(R1, E, AF.Exp, scale=-1.0)
            exp_ops = [i_ep, i_em, i_lnq, i_lna, i_inv]

            # act-set ordering: trig group, then exp group, then next trig
            for a_op in exp_ops:
                for b_op in trig_ops:
                    tile.add_dep_helper(a_op.ins, b_op.ins, reason="act-set order")
            for b_op in trig_ops:
                for pr in state["prev_ops"]:
                    tile.add_dep_helper(b_op.ins, pr.ins, reason="act order")

            nc.vector.tensor_tensor(R3, R3, R1, ALU.mult)        # t0 = b/|a|
            nc.vector.tensor_tensor(R5, R3, G, ALU.mult)         # t0*g
            nc.vector.scalar_tensor_tensor(R5, R5, -2.0, R3, ALU.mult, ALU.add)
            state["prev_t"] = R5
            state["prev_col"] = col + 1
            state["prev_ops"] = exp_ops

            if last:
                R4 = stage.tile(shp, BF16, tag=f"R4_{nst}")
                i_at = nc.scalar.activation(
                    R4, R5, AF.Arctan,
                    accum_out=acc_sb[:, col + 1:col + 2])
                for pr in exp_ops:
                    tile.add_dep_helper(i_at.ins, pr.ins, reason="act order")
                state["prev_t"] = None

        for bi, (q, st_lo, st_hi, col) in enumerate(BLOCKS):
            ps = ppool.tile([128, N_ST, 2 * OQ], F32, tag="ps")
            for st in range(st_lo, st_hi):
                for k2 in range(N_K2):
                    nc.tensor.matmul(
                        ps[:, st, :],
                        symt[k2][:, :, st * 128:(st + 1) * 128],
                        wt[q][k2 // 4][:, k2 % 4, :, :],
                        start=(k2 == 0), stop=(k2 == N_K2 - 1),
                        perf_mode=DR,
                    )
            emit_block(ps, st_lo, st_hi, col, last=(bi == len(BLOCKS) - 1))

        nc.sync.dma_start(out=acc[:, :], in_=acc_sb)

    nc.finalize()
    return nc


def _get_nc():
    global _nc_cache
    if _nc_cache is None:
        _nc_cache = _build_nc()
    return _nc_cache


def _host_prep(vis_states, weights):
    vis = np.asarray(vis_states).astype(np.float32)
    v = 2.0 * vis - 1.0
    vv = np.concatenate([v, v]).astype(ml_dtypes.float8_e4m3)  # +-1, exact
    w = np.asarray(weights)
    ws_r = (w.real.astype(np.float32).T * np.float32(SCALE)).astype(
        ml_dtypes.float8_e4m3)                                  # (4096, 1024)
    ws_i = (w.imag.astype(np.float32).T * np.float32(SCALE)).astype(
        ml_dtypes.float8_e4m3)
    # swizzle to [q, k2g, p, k2in, i, j, o] so each DMA chunk (q, k2g) is a
    # [128, 4096B] straight partition-major copy.
    def swz(ws):
        t = ws.reshape(4, 4, 2, 128, N_Q, OQ)      # [k2g, k2in, i, p, q, o]
        return np.transpose(t, (4, 0, 3, 1, 2, 5))  # [q, k2g, p, k2in, i, o]
    wq = np.empty((N_Q, 4, 128, 4, 2, 2, OQ), dtype=ml_dtypes.float8_e4m3)
    wq[..., 0, :] = swz(ws_r)
    wq[..., 1, :] = swz(ws_i)
    return v, vv, ws_r, ws_i, wq.reshape(2048, 4096)


def _reduce_acc(acc_arrays):
    tot_ln = 0.0
    tot_at = 0.0
    tot_g = 0.0
    tot_gh = 0.0
    for a in acc_arrays:
        a = a.astype(np.float64)
        tot_ln += a[:, 0::4].sum()
        tot_at += a[:, 1::4].sum()
        tot_g += a[:, 2::4].sum()
        tot_gh += a[:, 3::4].sum()
    n_counted = N_CORES * S_PER_CORE * OUP_N  # includes the phantom shift
    real = 0.5 * tot_ln - math.log(2.0) * n_counted
    imag = tot_at + math.pi * (2.0 * tot_gh - tot_g)
    return real, imag


def kernel(vis_states: np.ndarray, weights: np.ndarray) -> np.ndarray:
    global last_results
    v, vv, ws_r, ws_i, wq = _host_prep(vis_states, weights)

    in_maps = []
    for c in range(N_CORES):
        s0 = c * S_PER_CORE
        in_maps.append(
            {"vwin": np.ascontiguousarray(vv[s0:s0 + WIN]), "wq": wq}
        )

    nc = _get_nc()
    res = run_bass_kernel_spmd(nc, in_maps, core_ids=list(range(N_CORES)))
    last_results = res

    real, imag = _reduce_acc([r["acc"] for r in res.results])

    # subtract the phantom wrap-around shift 4095 (core 7 row 512), using
    # the SAME fp8-dequantized weights the device used.
    v4095 = np.concatenate([v, v])[4095:4095 + INP_N].astype(np.float64)
    pre_r = v4095 @ ws_r.astype(np.float64) / SCALE          # (1024,)
    pre_i = v4095 @ ws_i.astype(np.float64) / SCALE
    phantom = np.log(np.cosh(pre_r + 1j * pre_i)).sum()
    real -= phantom.real
    imag -= phantom.imag

    return np.array(real + 1j * imag, dtype=np.complex64)


# revision 14
# speedup vs baseline: 1.1081x; 1.1081x over previous
"""CpxRBM translation-invariant log-psi kernel for 8 Trainium2 NeuronCores.

Computes sum(log(cosh(sym @ W.T))) where sym is the (4095, 4096) matrix of
circular shifts of v = 2*vis_states - 1 and W is (1024, 4096) complex64.

v5 strategy (shift-sharded, 512 shifts/core, fp8 DoubleRow matmuls):
  - Host sends the per-core 4608-window of doubled v as fp8 {-1,+1} (exact)
    and the weights as fp8e4 scaled by 64 (power of two; undone for free via
    ACT affine scales).  Core 7's phantom 4096th shift (the wrap-around
    shift 4095) is subtracted EXACTLY on the host using the same
    fp8-dequantized weights.
  - Matmul: perf_mode=DoubleRow packs 2 fp8 weights/PE cell: each matmul
    contracts K=256 (two 128-row chunks) with a 128x1024-fp8 moving operand
    -> 256 matmuls x 512 out cols (216ns each, ~2x over bf16).  sym slices
    stationary, weights moving, fp32 PSUM accumulation.
  - OUTPUT-QUARTER-outer loop: each ~13.8us matmul phase consumes 2.1MB of
    weights.  Weights arrive as 16 x 525KB DMAs with 4KB-contiguous
    partition lines, alternating across BOTH HWDGE rings (sync + scalar) so
    the per-DMA fixed costs overlap; sym arrives as 4 x 575KB quad-DMAs
    interleaved ahead of the weight chunks they pace.
  - log(cosh(x+iy)) elementwise (x,y read straight from PSUM by ACT):
      sy = sin(y), cy = sin(pi/2 - |y|) = cos(y)   (direct: |y| <= ~3.6 and
        the HW sin table is accurate there - verified by probe)
      q = (e^x+e^-x)^2 - 4 sy^2 = |2cosh z|^2
      Re = 0.5*ln(q + 1e-6) - ln2        (ACT Ln accumulates row sums)
      Im = atan2(b, a) = atan(b/a) + pi*[a<0]*sign(b),
        a = t1*cy, b = t2*sy; 1/|a| via Exp/Ln (same table set), quadrant
        counts accumulate via DVE is_lt / is_ge accum_out columns.
        (HW arctan verified accurate over the full input range.)
    Two ACT table sets only; block i's Arctan rides in block i+1's trig
    residency.  DVE ops are emitted on the ln(q) critical path first.
  - Per-core output: (128, 16) fp32 accumulator columns; host reduces.
"""
import math
import numpy as np
import ml_dtypes
from contextlib import ExitStack

import concourse.bass as bass
import concourse.mybir as mybir
import concourse.tile as tile
from concourse import bacc
from concourse.bass_utils import run_bass_kernel_spmd
from concourse.hw_specs import get_activation_tables
import bass_rust as _bass_rust

F32 = mybir.dt.float32
BF16 = mybir.dt.bfloat16
FP8 = mybir.dt.float8e4
AF = mybir.ActivationFunctionType
ALU = mybir.AluOpType
DR = mybir.MatmulPerfMode.DoubleRow

PI = float(np.pi)
VIS_N = 4096
INP_N = 4096
OUP_N = 1024
N_CORES = 8
S_PER_CORE = 512
N_K2 = 16                  # 256-row contraction chunks
N_ST = 4                   # shift tiles of 128 per core
N_Q = 4                    # output quarters (phases/blocks)
OQ = OUP_N // N_Q          # 256 output cols per quarter
WIN = S_PER_CORE + INP_N   # 4608
SCALE = 64.0               # fp8 weight scale (power of 2)
EPS_Q = 1e-6               # Ln bias: absorbs fp32 rounding of q ~ 0
EPS_A = 1e-4               # Ln bias: absorbs bf16 rounding of |a| ~ 0
N_COLS = 16

_ALLOWED_SETS = {"natural_log_exp_and_others", "trig_and_small"}


class _Bacc(bacc.Bacc):
    def insert_act_table_loads(self):
        has_activation = any(
            isinstance(i, mybir.InstActivation)
            for b in self.main_func.blocks
            for i in b.instructions
        )
        if not has_activation:
            return
        tables = [
            (name, funcs if name in _ALLOWED_SETS else set())
            for name, funcs in get_activation_tables(self.m.arch).items()
        ]
        _bass_rust.insert_act_table_loads(self, tables)


_nc_cache = None
last_results = None


def _build_nc():
    nc = _Bacc("TRN2", target_bir_lowering=False, debug=False)

    vwin = nc.dram_tensor("vwin", [WIN], FP8, kind="ExternalInput")
    # weights, swizzled: row (q*4 + g)*128 + p holds the 4KB line
    # [k2in(4), i(2), j(2), o(256)] for partition p of DMA chunk (q, g).
    wq = nc.dram_tensor("wq", [2048, 4096], FP8, kind="ExternalInput")
    acc = nc.dram_tensor("acc", [128, N_COLS], F32, kind="ExternalOutput")

    with tile.TileContext(nc) as tc, ExitStack() as ctx:
        singles = ctx.enter_context(tc.tile_pool(name="singles", bufs=1))
        sympool = ctx.enter_context(tc.tile_pool(name="sympool", bufs=1))
        wpool = ctx.enter_context(tc.tile_pool(name="wpool", bufs=1))
        ppool = ctx.enter_context(tc.tile_pool(name="ppool", bufs=2, space="PSUM"))
        stage = ctx.enter_context(tc.tile_pool(name="stage", bufs=2))

        acc_sb = singles.tile([128, N_COLS], F32)
        half_pi = singles.tile([128, 1], F32)
        nc.vector.memset(half_pi, PI / 2.0)
        eps_q = singles.tile([128, 1], F32)
        nc.vector.memset(eps_q, EPS_Q)
        eps_a = singles.tile([128, 1], F32)
        nc.vector.memset(eps_a, EPS_A)

        # DMA streams on both HWDGE rings, interleaved in consumption order.
        # sym quads: symq[qd][p, k, i, s] = vwin[256*(4qd+k) + 128*i + p + s]
        symq = []
        for qd in range(4):
            t = sympool.tile([128, 4, 2, S_PER_CORE], FP8, tag=f"sym{qd}",
                             name=f"sym{qd}")
            symq.append(t)
        # weights: wt[q][g][p, k2in, i, jo]
        wt = [[None] * 4 for _ in range(N_Q)]
        for q in range(N_Q):
            for g in range(4):
                wt[q][g] = wpool.tile([128, 4, 2, 2 * OQ], FP8,
                                      tag=f"w{q}_{g}", name=f"w{q}_{g}")

        def sym_dma(eng, qd):
            eng.dma_start(
                out=symq[qd],
                in_=bass.AP(vwin, 1024 * qd,
                            [[1, 128], [256, 4], [128, 2], [1, S_PER_CORE]]),
            )

        def w_dma(eng, q, g):
            eng.dma_start(
                out=wt[q][g],
                in_=bass.AP(wq, (q * 4 + g) * 128 * 4096,
                            [[4096, 128], [1, 4096]]),
            )

        # ring schedules (needed-by order): sync gets even chunks, scalar odd
        sym_dma(nc.sync, 0)
        sym_dma(nc.scalar, 1)
        w_dma(nc.sync, 0, 0)
        w_dma(nc.scalar, 0, 1)
        sym_dma(nc.sync, 2)
        sym_dma(nc.scalar, 3)
        w_dma(nc.sync, 0, 2)
        w_dma(nc.scalar, 0, 3)
        for q in range(1, N_Q):
            for g in range(4):
                w_dma(nc.sync if g % 2 == 0 else nc.scalar, q, g)

        state = {"prev_t": None, "prev_col": None, "prev_ops": []}

        def emit_block(ps, col, last=False):
            """log(cosh) on ps ([128, 4, 512] fp32 psum, free cols =
            [re | im] halves of 256); accumulates into acc_sb columns
            col+0: sum ln q, col+1: sum atan, col+2: sum g, col+3: sum g&h."""
            xr = ps[:, :, 0:OQ]
            xi = ps[:, :, OQ:2 * OQ]
            shp = [128, N_ST, OQ]

            A = stage.tile(shp, F32, tag="A")    # sy
            B = stage.tile(shp, F32, tag="B")    # |y| -> sy^2 -> q
            Bc = stage.tile(shp, F32, tag="Bc")  # cy
            C = stage.tile(shp, F32, tag="C")    # e^x
            D = stage.tile(shp, F32, tag="D")    # e^-x -> t2
            Ft = stage.tile(shp, F32, tag="F")   # t1
            E = stage.tile(shp, F32, tag="E")    # t1^2, ln scratch
            R1 = stage.tile(shp, BF16, tag="R1")  # 1/|a|
            R2 = stage.tile(shp, BF16, tag="R2")  # a -> |a|
            R3 = stage.tile(shp, BF16, tag="R3")  # b -> t0
            R5 = stage.tile(shp, BF16, tag="R5")  # t''
            G = stage.tile(shp, BF16, tag="G")    # [a<0]
            H = stage.tile(shp, BF16, tag="H")    # g*[b>=0]

            # --- ACT trig phase (prev block's Arctan rides along) ---
            trig_ops = []
            if state["prev_t"] is not None:
                R4 = stage.tile(shp, BF16, tag="R4")
                i_at = nc.scalar.activation(
                    R4, state["prev_t"], AF.Arctan,
                    accum_out=acc_sb[:, state["prev_col"]:state["prev_col"] + 1],
                )
                trig_ops.append(i_at)
            i_sy = nc.scalar.activation(A, xi, AF.Sin, scale=1.0 / SCALE)
            i_ay = nc.scalar.activation(B, xi, AF.Abs, scale=1.0 / SCALE)
            i_cy = nc.scalar.activation(Bc, B, AF.Sin, bias=half_pi, scale=-1.0)
            trig_ops += [i_sy, i_ay, i_cy]
            # --- ACT exp phase ---
            i_ep = nc.scalar.activation(C, xr, AF.Exp, scale=1.0 / SCALE)
            i_em = nc.scalar.activation(D, xr, AF.Exp, scale=-1.0 / SCALE)

            # DVE, emitted on the ln(q) critical path first (in-order engine)
            nc.vector.tensor_tensor(B, A, A, ALU.mult)            # sy^2
            nc.vector.tensor_tensor(Ft, C, D, ALU.add)            # t1
            nc.vector.tensor_tensor(E, Ft, Ft, ALU.mult)          # t1^2
            nc.vector.scalar_tensor_tensor(B, B, -4.0, E, ALU.mult, ALU.add)
            i_lnq = nc.scalar.activation(
                E, B, AF.Ln, bias=eps_q,
                accum_out=acc_sb[:, col:col + 1])
            # off-path DVE while ACT runs lnq
            nc.vector.tensor_tensor(D, C, D, ALU.subtract)        # t2
            nc.vector.tensor_tensor(R2, Ft, Bc, ALU.mult)         # a (bf16)
            nc.vector.tensor_scalar(G, R2, 0.0, None, ALU.is_lt, ALU.add,
                                    accum_out=acc_sb[:, col + 2:col + 3])
            nc.vector.scalar_tensor_tensor(R2, R2, -1.0, R2, ALU.mult,
                                           ALU.max)               # |a|
            i_lna = nc.scalar.activation(E, R2, AF.Ln, bias=eps_a)
            nc.vector.tensor_tensor(R3, D, A, ALU.mult)           # b (bf16)
            nc.vector.scalar_tensor_tensor(
                H, R3, 0.0, G, ALU.is_ge, ALU.mult)               # g*[b>=0]
            nc.vector.tensor_scalar(H, H, 1.0, None, ALU.mult, ALU.add,
                                    accum_out=acc_sb[:, col + 3:col + 4])
            i_inv = nc.scalar.activation(R1, E, AF.Exp, scale=-1.0)
            nc.vector.tensor_tensor(R3, R3, R1, ALU.mult)         # t0
            nc.vector.tensor_tensor(R5, R3, G, ALU.mult)          # t0*g
            nc.vector.scalar_tensor_tensor(R5, R5, -2.0, R3, ALU.mult, ALU.add)

            exp_ops = [i_ep, i_em, i_lnq, i_lna, i_inv]
            for a_op in exp_ops:
                for b_op in trig_ops:
                    tile.add_dep_helper(a_op.ins, b_op.ins, reason="act-set order")
            for b_op in trig_ops:
                for pr in state["prev_ops"]:
                    tile.add_dep_helper(b_op.ins, pr.ins, reason="act order")

            state["prev_t"] = R5
            state["prev_col"] = col + 1
            state["prev_ops"] = exp_ops

            if last:
                R4 = stage.tile(shp, BF16, tag="R4")
                i_at = nc.scalar.activation(
                    R4, R5, AF.Arctan,
                    accum_out=acc_sb[:, col + 1:col + 2])
                for pr in exp_ops:
                    tile.add_dep_helper(i_at.ins, pr.ins, reason="act order")
                state["prev_t"] = None

        for q in range(N_Q):
            ps = ppool.tile([128, N_ST, 2 * OQ], F32, tag="ps")
            for st in range(N_ST):
                for k2 in range(N_K2):
                    nc.tensor.matmul(
                        ps[:, st, :],
                        symq[k2 // 4][:, k2 % 4, :, st * 128:(st + 1) * 128],
                        wt[q][k2 // 4][:, k2 % 4, :, :],
                        start=(k2 == 0), stop=(k2 == N_K2 - 1),
                        perf_mode=DR,
                    )
            emit_block(ps, 4 * q, last=(q == N_Q - 1))

        nc.sync.dma_start(out=acc[:, :], in_=acc_sb)

    nc.finalize()
    return nc


def _get_nc():
    global _nc_cache
    if _nc_cache is None:
        _nc_cache = _build_nc()
    return _nc_cache


def _host_prep(vis_states, weights):
    vis = np.asarray(vis_states).astype(np.float32)
    v = 2.0 * vis - 1.0
    vv = np.concatenate([v, v]).astype(ml_dtypes.float8_e4m3)  # +-1, exact
    w = np.asarray(weights)
    ws_r = (w.real.astype(np.float32).T * np.float32(SCALE)).astype(
        ml_dtypes.float8_e4m3)                                  # (4096, 1024)
    ws_i = (w.imag.astype(np.float32).T * np.float32(SCALE)).astype(
        ml_dtypes.float8_e4m3)
    # swizzle to [q, g, p, k2in, i, j, o] so each DMA chunk (q, g) is a
    # [128, 4096B] straight partition-major copy.
    def swz(ws):
        t = ws.reshape(4, 4, 2, 128, N_Q, OQ)      # [g, k2in, i, p, q, o]
        return np.transpose(t, (4, 0, 3, 1, 2, 5))  # [q, g, p, k2in, i, o]
    wq = np.empty((N_Q, 4, 128, 4, 2, 2, OQ), dtype=ml_dtypes.float8_e4m3)
    wq[..., 0, :] = swz(ws_r)
    wq[..., 1, :] = swz(ws_i)
    return v, vv, ws_r, ws_i, wq.reshape(2048, 4096)


def _reduce_acc(acc_arrays):
    tot_ln = tot_at = tot_g = tot_gh = 0.0
    for a in acc_arrays:
        a = a.astype(np.float64)
        tot_ln += a[:, 0::4].sum()
        tot_at += a[:, 1::4].sum()
        tot_g += a[:, 2::4].sum()
        tot_gh += a[:, 3::4].sum()
    n_counted = N_CORES * S_PER_CORE * OUP_N  # includes the phantom shift
    real = 0.5 * tot_ln - math.log(2.0) * n_counted
    imag = tot_at + math.pi * (2.0 * tot_gh - tot_g)
    return real, imag


def kernel(vis_states: np.ndarray, weights: np.ndarray) -> np.ndarray:
    global last_results
    v, vv, ws_r, ws_i, wq = _host_prep(vis_states, weights)

    in_maps = []
    for c in range(N_CORES):
        s0 = c * S_PER_CORE
        in_maps.append(
            {"vwin": np.ascontiguousarray(vv[s0:s0 + WIN]), "wq": wq}
        )

    nc = _get_nc()
    res = run_bass_kernel_spmd(nc, in_maps, core_ids=list(range(N_CORES)))
    last_results = res

    real, imag = _reduce_acc([r["acc"] for r in res.results])

    # subtract the phantom wrap-around shift 4095 (core 7 row 512), using
    # the SAME fp8-dequantized weights the device used.
    v4095 = np.concatenate([v, v])[4095:4095 + INP_N].astype(np.float64)
    pre_r = v4095 @ ws_r.astype(np.float64) / SCALE          # (1024,)
    pre_i = v4095 @ ws_i.astype(np.float64) / SCALE
    phantom = np.log(np.cosh(pre_r + 1j * pre_i)).sum()
    real -= phantom.real
    imag -= phantom.imag

    return np.array(real + 1j * imag, dtype=np.complex64)
